# revision 7
# baseline (speedup 1.0000x reference)
"""Trainium2 Bass kernel for AgreementRouting (dynamic routing / capsule-style).

Full-input contract: kernel(u_predict[64,2048,32,16] f32, b[2048,32] f32) -> v[64,32,16] f32.
Internally shards batch (64) across 8 NeuronCores (8 batch elems per core).

Per-core algorithm (B_loc=8, L=2048, H=32, D=16, HD=512), f16 compute with
f32 accumulation. Both u layouts (natural + transposed) are f16 in SBUF with
a 5-deep wave-rotated residency, giving a continuous cross-batch software
pipeline (no group barriers) that keeps the PE array warm:
  load:  u fp32 HBM -> SBUF f16 via gpsimd casting DMA (16 tiles/b)
  uT:    regular matmuls vs f16 identity rhs (f16 FWL weights) -> f32 PSUM,
         evacuated by the Scalar engine with a free f32->f16 cast
  per routing iteration:
    agree:   upd[l,h] += sum_hd uT[hd,l]*V[hd,h]; uT slices as f16 FWL
             weights, block-diag V_mat f16 rhs; PSUM bank per in-flight b
    softmax: bb += upd (DVE); exp (ACT); reduce/recip/mult (DVE) -> c f16
    ws:      O2[hd,h'] = sum_l u[l,hd]*c[l,h'] with nat f16 weights, c f16
             rhs; s extracted via DVE mask-multiply + grouped reduce
    squash:  factor via tiny f32 matmuls (h_mat/ind_t) in hd-partition
             layout; V_mat built by ACT scaled copies (per-partition scale)
  output: final v^T [128,4] fp32 DMA'd per batch elem
"""

import sys
import os

sys.path.insert(0, "/opt/trn_rl_repo")

import numpy as np
from contextlib import ExitStack

B, L, H, D = 64, 2048, 32, 16
NCORES = 8
BLOC = B // NCORES  # 8
HD = H * D  # 512
NT = L // 128  # 16 l-chunks
NKC = HD // 128  # 4 hd chunks
NITER = 3
EPS = 1e-8

_NC_CACHE = {}
LAST_EXEC_NS = None
LAST_RESULTS = None
LAST_TRACE_DIR = None
_TRACE = False


def _consts():
    p = np.arange(128)
    # ind_mask[p, H*c + h] = 1 iff h == 8c + p//16   (h-index of hd = 128c + p)
    ind_mask = np.zeros((128, NKC * H), np.float32)
    for c in range(NKC):
        ind_mask[p, H * c + 8 * c + p // 16] = 1.0
    # h_mat[p, g] = 1 iff p//16 == g
    h_mat = (p[:, None] // 16 == np.arange(8)[None, :]).astype(np.float32)
    ind_t = np.ascontiguousarray(h_mat.T)  # [8, 128]
    ident16 = np.eye(128, dtype=np.float16)
    return {
        "ind_mask": ind_mask,
        "h16": h_mat,
        "it16": ind_t,
        "ident16": ident16,
    }


def _emit(ctx, tc, t_in, t_out):
    import concourse.mybir as mybir

    nc = tc.nc
    f32 = mybir.dt.float32
    f16 = mybir.dt.float16
    f8 = mybir.dt.float8e4
    AF = mybir.ActivationFunctionType
    ALU = mybir.AluOpType
    AX = mybir.AxisListType

    u_ap = t_in["u"]
    b_ap = t_in["b"]
    im_ap = t_in["ind_mask"]
    h_ap = t_in["h16"]
    it_ap = t_in["it16"]
    id_ap = t_in["ident16"]
    vout_ap = t_out["v_out"]

    WAVE = 5  # b's resident simultaneously (SBUF-limited)
    cpool = ctx.enter_context(tc.tile_pool(name="cpool", bufs=1))
    # u f16, both layouts, wave-rotated residency
    p_nat = ctx.enter_context(tc.tile_pool(name="p_nat", bufs=WAVE * NT))
    p_uT = ctx.enter_context(tc.tile_pool(name="p_uT", bufs=WAVE * NKC))
    p_bb = ctx.enter_context(tc.tile_pool(name="p_bb", bufs=BLOC))
    p_soft = ctx.enter_context(tc.tile_pool(name="p_soft", bufs=4))
    p_ct = ctx.enter_context(tc.tile_pool(name="p_ct", bufs=4))
    p_small = ctx.enter_context(tc.tile_pool(name="p_small", bufs=8))
    p_prod = ctx.enter_context(tc.tile_pool(name="p_prod", bufs=4))
    p_s4 = ctx.enter_context(tc.tile_pool(name="p_s4", bufs=12))
    p_sq = ctx.enter_context(tc.tile_pool(name="p_sq", bufs=24))
    p_vm = ctx.enter_context(tc.tile_pool(name="p_vm", bufs=3 * NKC))
    # PSUM: 2 + 2 + 3 + 1 = 8 banks
    ps_tr = ctx.enter_context(tc.tile_pool(name="ps_tr", bufs=2, space="PSUM"))
    ps_o = ctx.enter_context(tc.tile_pool(name="ps_o", bufs=2, space="PSUM"))
    ps_upd = ctx.enter_context(tc.tile_pool(name="ps_upd", bufs=3, space="PSUM"))
    ps_t = ctx.enter_context(tc.tile_pool(name="ps_t", bufs=1, space="PSUM"))

    # ---- constants
    im_t = cpool.tile([128, NKC * H], f32, name="im_t")
    nc.sync.dma_start(im_t[:], im_ap)
    h_t = cpool.tile([128, 8], f32, name="h_t")
    nc.sync.dma_start(h_t[:], h_ap)
    it_t = cpool.tile([8, 128], f32, name="it_t")
    nc.sync.dma_start(it_t[:], it_ap)
    id_t = cpool.tile([128, 128], f16, name="id_t")
    nc.sync.dma_start(id_t[:], id_ap)
    bin_t = cpool.tile([128, NT * H], f32, name="bin_t")
    nc.sync.dma_start(
        bin_t[:].rearrange("p (t h) -> p t h", t=NT),
        b_ap.rearrange("(t p) h -> p t h", p=128),
    )

    # ---- c0 = softmax(b) over h (shared across batch; logits bounded)
    e0 = cpool.tile([128, NT * H], f32, name="e0")
    nc.scalar.activation(e0[:], bin_t[:], AF.Exp)
    z0 = p_small.tile([128, NT], f32, name="z0", tag="small")
    nc.vector.reduce_sum(z0[:], e0[:].rearrange("p (t h) -> p t h", t=NT), AX.X)
    r0 = p_small.tile([128, NT], f32, name="r0", tag="small")
    nc.vector.reciprocal(r0[:], z0[:])
    c0 = cpool.tile([128, NT * H], f16, name="c0")
    nc.vector.tensor_tensor(
        c0[:].rearrange("p (t h) -> p t h", t=NT),
        e0[:].rearrange("p (t h) -> p t h", t=NT),
        r0[:].unsqueeze(2).broadcast_to((128, NT, H)),
        ALU.mult,
    )

    st = {}  # per-b state

    def emit_load(b):
        nat = []
        for t in range(NT):
            s16 = p_nat.tile([128, HD], f16, name="s16", tag="nat")
            nc.gpsimd.dma_start(
                s16[:],
                u_ap[b, 128 * t : 128 * (t + 1)].rearrange("l h d -> l (h d)"),
            )
            nat.append(s16)
        st[b] = {"nat": nat}

    def emit_prep(b):
        """uT via regular matmuls vs identity; ACT evacuates with f32->fp8 cast."""
        nat = st[b]["nat"]
        uT = []
        for k in range(NKC):
            uTk = p_uT.tile([128, L], f16, name="uTk", tag="uT")
            uT.append(uTk)
        for k in range(NKC):
            for tq in range(NT // 4):
                ptr = ps_tr.tile(
                    [128, 4 * 128], f32, name="ptr", tag="ptr", padded_shape=[128, 512]
                )
                for j in range(4):
                    nc.tensor.matmul(
                        ptr[:, 128 * j : 128 * (j + 1)],
                        nat[4 * tq + j][:, 128 * k : 128 * (k + 1)],
                        id_t[:],
                        start=True,
                        stop=True,
                    )
                nc.scalar.activation(
                    uT[k][:, 512 * tq : 512 * (tq + 1)], ptr[:], AF.Copy
                )
        st[b]["uT"] = uT

    def emit_ws(b, c_tile, last):
        """weighted-sum via u-as-weights: O2[hd, h'] = sum_l u[l, hd] c[l, h'],
        then mask-multiply + grouped reduce extracts s into SBUF."""
        nat = st[b]["nat"]
        O2 = ps_o.tile([128, NKC * H], f32, name="O2", tag="O", padded_shape=[128, 512])
        cv = c_tile[:].rearrange("p (t h) -> p t h", t=NT)
        for k in range(NKC):
            for t in range(NT):
                nc.tensor.matmul(
                    O2[:, H * k : H * (k + 1)],
                    nat[t][:, 128 * k : 128 * (k + 1)],
                    cv[:, t, :],
                    start=(t == 0),
                    stop=(t == NT - 1),
                )
        prod = p_prod.tile([128, NKC * H], f32, name="prod", tag="prod")
        s_sb = p_s4.tile([128, NKC], f32, name="s_sb", tag="s4")
        nc.vector.tensor_tensor(prod[:], O2[:], im_t[:], ALU.mult)
        nc.vector.reduce_sum(
            s_sb[:], prod[:].rearrange("p (k h) -> p k h", k=NKC), AX.X
        )
        s2 = p_s4.tile([128, NKC], f32, name="s2", tag="s4")
        nc.scalar.square(s2[:], s_sb[:])
        # sq^T[g, c] = ||s_h||^2 for h = 8c + g
        sqT = ps_t.tile([8, NKC], f32, name="sqT", tag="pt", padded_shape=[128, 512])
        nc.tensor.matmul(sqT[:], h_t[:], s2[:], start=True, stop=True)
        st[b]["s_sb"] = s_sb
        st[b]["sqT"] = sqT
        st[b]["last"] = last

    def emit_squash(b):
        """squash factor f = q/(1+q)/sqrt(q+eps); vT; V_mat (or output DMA)."""
        sqT = st[b]["sqT"]
        s_sb = st[b]["s_sb"]
        last = st[b]["last"]
        t1 = p_sq.tile([8, NKC], f32, name="t1", tag="sq")
        nc.vector.tensor_scalar_add(t1[:], sqT[:], 1.0)
        r1 = p_sq.tile([8, NKC], f32, name="r1", tag="sq")
        nc.vector.reciprocal(r1[:], t1[:])
        teps = p_sq.tile([8, NKC], f32, name="teps", tag="sq")
        nc.vector.tensor_scalar_add(teps[:], sqT[:], EPS)
        rt = p_sq.tile([8, NKC], f32, name="rt", tag="sq")
        nc.scalar.activation(rt[:], teps[:], AF.Sqrt)
        r2 = p_sq.tile([8, NKC], f32, name="r2", tag="sq")
        nc.vector.reciprocal(r2[:], rt[:])
        g1 = p_sq.tile([8, NKC], f32, name="g1", tag="sq")
        nc.vector.tensor_tensor(g1[:], sqT[:], r1[:], ALU.mult)
        fT = p_sq.tile([8, NKC], f32, name="fT", tag="sq")
        nc.vector.tensor_tensor(fT[:], g1[:], r2[:], ALU.mult)
        # expand f to hd-partition layout: fexp[p, c] = f[8c + p//16]
        fexp = ps_t.tile([128, NKC], f32, name="fexp", tag="pt", padded_shape=[128, 512])
        nc.tensor.matmul(fexp[:], it_t[:], fT[:], start=True, stop=True)
        vT = p_s4.tile([128, NKC], f32, name="vT", tag="s4")
        nc.vector.tensor_tensor(vT[:], s_sb[:], fexp[:], ALU.mult)
        if last:
            nc.sync.dma_start(
                vout_ap[b].rearrange("h d -> (h d)").rearrange("(c p) -> p c", p=128),
                vT[:],
            )
            st[b]["vms"] = None
            return
        # V_mat: vm_k[p, h] = vT[p, k] * ind_mask[p, 32k + h]  (ACT scaled copy)
        vms = []
        for c in range(NKC):
            vm_c = p_vm.tile([128, H], f16, name="vmc", tag="vm")
            nc.scalar.activation(
                vm_c[:],
                im_t[:, H * c : H * (c + 1)],
                AF.Copy,
                scale=vT[:, c : c + 1],
            )
            vms.append(vm_c)
        st[b]["vms"] = vms

    def emit_agree(b):
        """agreement matmuls: uT fp8 FWL weights x block-diag V f16 rhs."""
        uT = st[b]["uT"]
        vms = st[b]["vms"]
        upd = ps_upd.tile(
            [128, NT * H], f32, name="upd", tag="upd", padded_shape=[128, 512]
        )
        for t in range(NT):
            for k in range(NKC):
                nc.tensor.matmul(
                    upd[:, H * t : H * (t + 1)],
                    uT[k][:, 128 * t : 128 * (t + 1)],
                    vms[k][:],
                    start=(k == 0),
                    stop=(k == NKC - 1),
                )
        st[b]["upd"] = upd

    def emit_softmax(b, first):
        """bb += upd; c = softmax(bb) over h -> fresh f16 c tile."""
        upd = st[b]["upd"]
        if first:
            bb_t = p_bb.tile([128, NT * H], f32, name="bbt", tag="bb")
            nc.vector.tensor_tensor(bb_t[:], bin_t[:], upd[:], ALU.add)
            st[b]["bb"] = bb_t
        else:
            bb_t = st[b]["bb"]
            nc.vector.tensor_tensor(bb_t[:], bb_t[:], upd[:], ALU.add)
        e = p_soft.tile([128, NT * H], f32, name="e", tag="soft")
        nc.scalar.activation(e[:], bb_t[:], AF.Exp)
        z = p_small.tile([128, NT], f32, name="z", tag="small")
        nc.vector.reduce_sum(z[:], e[:].rearrange("p (t h) -> p t h", t=NT), AX.X)
        r = p_small.tile([128, NT], f32, name="r", tag="small")
        nc.vector.reciprocal(r[:], z[:])
        c_t = p_ct.tile([128, NT * H], f16, name="ct", tag="softc")
        nc.vector.tensor_tensor(
            c_t[:].rearrange("p (t h) -> p t h", t=NT),
            e[:].rearrange("p (t h) -> p t h", t=NT),
            r[:].unsqueeze(2).broadcast_to((128, NT, H)),
            ALU.mult,
        )
        st[b]["c"] = c_t

    # ---- schedule: skewed software pipeline across b (wave-rotated residency).
    # Emitting b's stage s only after b-1's stage s+1 keeps every pool-slot
    # WAR edge pointing backward in pipeline time (no dependency cycles).
    def emit_stage(b, s):
        if s == 0:
            emit_load(b)
        elif s == 1:
            emit_prep(b)
        elif s == 2:
            emit_ws(b, c0, False)
            emit_squash(b)
        else:
            it = s - 3
            emit_agree(b)
            emit_softmax(b, first=(it == 0))
            emit_ws(b, st[b]["c"], it == NITER - 1)
            emit_squash(b)

    NST = 3 + NITER
    for step in range(NST + BLOC - 1):
        for b in range(BLOC):
            s = step - b
            if 0 <= s < NST:
                emit_stage(b, s)


def _get_nc():
    if "nc" in _NC_CACHE:
        return _NC_CACHE["nc"]
    from concourse import bacc
    import concourse.tile as tile
    import concourse.mybir as mybir

    f32 = mybir.dt.float32
    f16 = mybir.dt.float16
    nc = bacc.Bacc("TRN2", target_bir_lowering=False, debug=False)
    t_in = {}
    in_shapes = {
        "u": ([BLOC, L, H, D], f32),
        "b": ([L, H], f32),
        "ind_mask": ([128, NKC * H], f32),
        "h16": ([128, 8], f32),
        "it16": ([8, 128], f32),
        "ident16": ([128, 128], f16),
    }
    for name, (shape, dt_) in in_shapes.items():
        t_in[name] = nc.dram_tensor(name, shape, dt_, kind="ExternalInput").ap()
    vout = nc.dram_tensor("v_out", [BLOC, H, D], f32, kind="ExternalOutput").ap()

    with tile.TileContext(nc) as tc:
        with ExitStack() as ctx:
            _emit(ctx, tc, t_in, {"v_out": vout})
    nc.compile()
    _NC_CACHE["nc"] = nc
    return nc


def kernel(u_predict, b):
    global LAST_EXEC_NS, LAST_RESULTS
    u = np.ascontiguousarray(np.asarray(u_predict, dtype=np.float32))
    bq = np.ascontiguousarray(np.asarray(b, dtype=np.float32))
    assert u.shape == (B, L, H, D), u.shape
    assert bq.shape == (L, H), bq.shape

    nc = _get_nc()
    consts = _consts()
    in_maps = []
    for i in range(NCORES):
        m = {"u": np.ascontiguousarray(u[i * BLOC : (i + 1) * BLOC]), "b": bq}
        m.update(consts)
        in_maps.append(m)

    from concourse.bass_utils import run_bass_kernel_spmd

    global LAST_TRACE_DIR
    kw = {}
    if _TRACE:
        import tempfile

        LAST_TRACE_DIR = tempfile.mkdtemp(prefix="bass_trace_")
        kw["tmpdir"] = LAST_TRACE_DIR
    res = run_bass_kernel_spmd(nc, in_maps, list(range(NCORES)), trace=_TRACE, **kw)
    LAST_EXEC_NS = res.exec_time_ns
    LAST_RESULTS = res
    out = np.concatenate([r["v_out"] for r in res.results], axis=0)
    return out.astype(np.float32)
